# revision 3
# baseline (speedup 1.0000x reference)
"""Trainium2 Bass kernel for sliding-window GQA attention block (v3).

Reference computation (B=2, S=4096, DIM=1024, H=16 q-heads, KV=2 kv-heads,
D=64, W=256 window):
    q = x@Wq + bq ; k = x@Wk + bk ; v = x@Wv + bv        (GQA repeat kv x8)
    local attention: query t attends keys [t-128, t+128) (zero-padded edges,
    no 1/sqrt(d) scaling), softmax, out = probs@v
    y = out@Wo + bo
Sharding: 8 cores = batch(2) x seq-quarter(4); each core computes 1024 query
rows end-to-end from a 1280-row haloed x slice; bo added on host.

v3 changes (vs v2, 146us):
  - all epilogue constants are HOST inputs: bkm = outer(bk, ind) and
    bvm = outer-ish(bv, ind) replace on-chip partition_broadcasts (the
    gpsimd lib-load + broadcast chain cost ~8us of head serialization);
    halo columns of x are zero, so the projections are already zero there
    and a single tensor_add fuses bias+halo-zeroing.  Band masks mA/mB are
    host inputs too.
  - PSUM is managed as four [128,1024] bank-PAIRS.  Score j-chunks put the
    two kv-halves in one pair -> ONE exp ACTIVATE per chunk ([128,1024] at
    (N+352)/1.2 ns amortizes the ACT fixed cost), one recip per (mt,gg),
    two normalize STTs per (mt,gg), one out copy + one out DMA per qtile.
  - band masks: gg0 chunks use gpsimd affine_select in-place on p2 (gpsimd
    is otherwise idle in the loop), gg1 chunks use DVE tensor_mul.
  - head: DMA descriptors are ordered so wk/wv land first and xT/wq stream
    round-robin on the three DMA queues (sync/scalar/gpsimd); Q m0-3 run
    k-major DMA-gated from ~5us; K/V0-2 follow; Q m4-7 and V3-9 are
    emitted as PE filler inside the first loop iterations.  The ACT exp
    table is preloaded with a dummy activation at t=0.
"""

import functools
import numpy as np

B, S, DIM = 2, 4096, 1024
H, KV, D = 16, 2, 64
W, HW = 256, 128
NCORES = 8
QT = 4           # sequence quarters
T = S // QT      # 1024 query rows per core
TH = T + 2 * HW  # 1280 haloed rows
NU = TH // 128   # 10 key/value u-tiles
KVD = KV * D     # 128


@functools.lru_cache(maxsize=1)
def _build_nc():
    import concourse.bacc as bacc
    import concourse.tile as tile
    from concourse import mybir

    f32 = mybir.dt.float32
    bf16 = mybir.dt.bfloat16
    Exp = mybir.ActivationFunctionType.Exp
    Identity = mybir.ActivationFunctionType.Identity
    MUL = mybir.AluOpType.mult

    nc = bacc.Bacc("TRN2", target_bir_lowering=False, debug=False)

    xTp = nc.dram_tensor("xTp", [128, 8 * TH], bf16, kind="ExternalInput")
    wqp = nc.dram_tensor("wqp", [128, 8 * DIM], bf16, kind="ExternalInput")
    wkp = nc.dram_tensor("wkp", [128, 8 * KVD], bf16, kind="ExternalInput")
    wvp = nc.dram_tensor("wvp", [128, 8 * KVD], bf16, kind="ExternalInput")
    wop = nc.dram_tensor("wop", [128, 8 * DIM], bf16, kind="ExternalInput")
    bqc = nc.dram_tensor("bqc", [128, 8], f32, kind="ExternalInput")
    bkmp = nc.dram_tensor("bkmp", [128, TH], bf16, kind="ExternalInput")
    bvmp = nc.dram_tensor("bvmp", [128, TH], bf16, kind="ExternalInput")
    mAp = nc.dram_tensor("mAp", [128, 1024], bf16, kind="ExternalInput")
    mBp = nc.dram_tensor("mBp", [128, 1024], bf16, kind="ExternalInput")
    out = nc.dram_tensor("out", [T, DIM], bf16, kind="ExternalOutput")

    with tile.TileContext(nc) as tc:
        with tc.tile_pool(name="const", bufs=1) as const, \
             tc.tile_pool(name="w", bufs=1) as wpool, \
             tc.tile_pool(name="act", bufs=1) as actp, \
             tc.tile_pool(name="attn", bufs=2) as attnp, \
             tc.tile_pool(name="ps", bufs=4, space="PSUM") as ps:

            # ---- ACT exp-table preload: dummy exp at t=0 so the ~2.7us
            # table load overlaps the DMA phase instead of the first scores.
            dum = const.tile([1, 2], f32, tag="dum")
            dumo = const.tile([1, 2], f32, tag="dumo")
            nc.vector.memset(dum, 0.0)
            nc.scalar.activation(out=dumo, in_=dum, func=Exp)

            # ---- SBUF tiles -----------------------------------------------
            xT_sb = wpool.tile([128, 8 * TH], bf16, tag="xT")
            wq_sb = wpool.tile([128, 8 * DIM], bf16, tag="wq")
            wk_sb = wpool.tile([128, 8 * KVD], bf16, tag="wk")
            wv_sb = wpool.tile([128, 8 * KVD], bf16, tag="wv")
            wo_sb = wpool.tile([128, 8 * DIM], bf16, tag="wo")
            bq_sb = const.tile([128, 8], f32, tag="bq")
            bkm_sb = const.tile([128, TH], bf16, tag="bkm")
            bvm_sb = const.tile([128, TH], bf16, tag="bvm")
            mA = const.tile([128, 1024], bf16, tag="mA")
            mB = const.tile([128, 1024], bf16, tag="mB")

            # ---- DMA schedule: wk/wv tiny and first (K/V never wait);
            # xT/wq round-robin; bias/mask consts next; wo last (needed
            # only by the first out-projection ~40us in).
            sq, sc, gq = nc.sync, nc.scalar, nc.gpsimd
            sc.dma_start(out=wk_sb, in_=wkp[:, :])
            sc.dma_start(out=wv_sb, in_=wvp[:, :])
            xd = lambda k: (xT_sb[:, k * TH:(k + 1) * TH],
                            xTp[:, k * TH:(k + 1) * TH])
            wd = lambda k2: (wq_sb[:, k2 * 2048:(k2 + 1) * 2048],
                             wqp[:, k2 * 2048:(k2 + 1) * 2048])
            o, i = xd(0); sq.dma_start(out=o, in_=i)
            o, i = wd(0); gq.dma_start(out=o, in_=i)
            o, i = xd(1); sc.dma_start(out=o, in_=i)
            o, i = xd(2); sq.dma_start(out=o, in_=i)
            o, i = wd(1); gq.dma_start(out=o, in_=i)
            o, i = xd(3); sc.dma_start(out=o, in_=i)
            o, i = xd(4); sq.dma_start(out=o, in_=i)
            o, i = wd(2); gq.dma_start(out=o, in_=i)
            o, i = xd(5); sc.dma_start(out=o, in_=i)
            o, i = xd(6); sq.dma_start(out=o, in_=i)
            o, i = wd(3); gq.dma_start(out=o, in_=i)
            o, i = xd(7); sc.dma_start(out=o, in_=i)
            sc.dma_start(out=bq_sb, in_=bqc[:, :])
            sq.dma_start(out=bkm_sb, in_=bkmp[:, :])
            gq.dma_start(out=bvm_sb, in_=bvmp[:, :])
            sq.dma_start(out=mA, in_=mAp[:, :])
            gq.dma_start(out=mB, in_=mBp[:, :])
            sq.dma_start(out=wo_sb[:, 0:4096], in_=wop[:, 0:4096])
            gq.dma_start(out=wo_sb[:, 4096:8192], in_=wop[:, 4096:8192])

            # ---- activations / attention SBUF -----------------------------
            qT_sb = [actp.tile([128, 4 * T], bf16, tag=f"qT{g}", name=f"qT{g}")
                     for g in range(2)]
            kT_sb = actp.tile([128, TH], bf16, tag="kT")
            v_sb = actp.tile([128, NU * 256], bf16, tag="V")
            v_view = v_sb.rearrange("p (u g c) -> p u g c", u=NU, g=2)
            nc.vector.memset(v_view[:, :, :, 0:64], 1.0)
            bvm_v = bvm_sb.rearrange("p (u g d) -> p u g d", u=NU, g=2)
            attnT = actp.tile([128, 8 * T], bf16, tag="attnT")
            qvs = [qT_sb[g].rearrange("p (i t) -> p i t", i=4) for g in range(2)]
            fz = nc.gpsimd.to_reg(0.0)

            # ---- Q projection subgroup: 2 m-tiles, k-inner, bias copy into
            # qT (even m on ACT, odd m on DVE).  m in 0..7; qT group g=m//4.
            def q_sub(ms):
                prs = {m: ps.tile([128, 1024], f32, tag="P", bufs=4,
                                  name=f"qp{m}") for m in ms}
                for k in range(8):
                    for m in ms:
                        for n in range(2):
                            nc.tensor.matmul(
                                out=prs[m][:, n * 512:(n + 1) * 512],
                                lhsT=wq_sb[:, k * DIM + m * 128:
                                           k * DIM + (m + 1) * 128],
                                rhs=xT_sb[:, k * TH + HW + n * 512:
                                          k * TH + HW + (n + 1) * 512],
                                start=(k == 0), stop=(k == 7))
                for m in ms:
                    dst = qT_sb[m // 4][:, (m % 4) * T:(m % 4) * T + 1024]
                    if m % 2 == 0:
                        nc.scalar.activation(out=dst, in_=prs[m],
                                             func=Identity,
                                             bias=bq_sb[:, m:m + 1], scale=1.0)
                    else:
                        nc.vector.tensor_scalar_add(out=dst, in0=prs[m],
                                                    scalar1=bq_sb[:, m:m + 1])

            # ---- K projection: kv*64+d on partitions, token on free.
            # bkm = outer(bk, ind) fuses bias + halo masking into one add.
            def k_proj():
                kp01 = ps.tile([128, 1024], f32, tag="P", bufs=4, name="kp01")
                kp2 = ps.tile([128, 1024], f32, tag="P", bufs=4, name="kp2")
                for k in range(8):
                    for c in range(2):
                        nc.tensor.matmul(
                            out=kp01[:, c * 512:(c + 1) * 512],
                            lhsT=wk_sb[:, k * KVD:(k + 1) * KVD],
                            rhs=xT_sb[:, k * TH + c * 512:k * TH + (c + 1) * 512],
                            start=(k == 0), stop=(k == 7))
                    nc.tensor.matmul(
                        out=kp2[:, 0:256],
                        lhsT=wk_sb[:, k * KVD:(k + 1) * KVD],
                        rhs=xT_sb[:, k * TH + 1024:k * TH + 1280],
                        start=(k == 0), stop=(k == 7))
                nc.vector.tensor_add(out=kT_sb[:, 0:1024], in0=kp01,
                                     in1=bkm_sb[:, 0:1024])
                nc.vector.tensor_add(out=kT_sb[:, 1024:1280], in0=kp2[:, 0:256],
                                     in1=bkm_sb[:, 1024:1280])

            # ---- V projection u-tile: keys on partitions; layout per
            # kv-half g: [ones(64) | V(64)] so flipped probs@[1|V] emits the
            # softmax denominator replicated on partitions 0-63.
            def v_proj(ut):
                vp = ps.tile([128, 1024], f32, tag="P", bufs=4, name="vp")
                for k in range(8):
                    nc.tensor.matmul(
                        out=vp[:, 0:128],
                        lhsT=xT_sb[:, k * TH + ut * 128:k * TH + (ut + 1) * 128],
                        rhs=wv_sb[:, k * KVD:(k + 1) * KVD],
                        start=(k == 0), stop=(k == 7))
                nc.vector.tensor_add(
                    out=v_view[:, ut, :, 64:128],
                    in0=vp[:, 0:128].rearrange("p (g c) -> p g c", g=2),
                    in1=bvm_v[:, ut])

            # ---- scores j-chunk: both kv-halves into one PSUM pair, one
            # exp over [128,1024]; band mask on gpsimd (gg0, in-place
            # affine_select) or DVE (gg1, tensor_mul with host mask).
            def scores_pair(mt, gg, j):
                qcol = mt * 128
                sp = ps.tile([128, 1024], f32, tag="P", bufs=4, name="sp")
                for h in range(2):
                    nc.tensor.matmul(
                        out=sp[:, h * 512:(h + 1) * 512],
                        lhsT=kT_sb[h * 64:(h + 1) * 64,
                                   qcol + j * 128:qcol + (j + 1) * 128],
                        rhs=qvs[gg][h * 64:(h + 1) * 64, :, qcol:qcol + 128],
                        start=True, stop=True,
                        tile_position=(64 * h, 0))
                p2 = attnp.tile([128, 1024], bf16, tag="p2", bufs=10, name="p2")
                nc.scalar.activation(out=p2, in_=sp, func=Exp)
                if j != 1:
                    if gg == 0:
                        pv8 = p2.rearrange("p (g c) -> p g c", g=8)
                        if j == 0:
                            nc.gpsimd.affine_select(
                                out=pv8, in_=pv8,
                                compare_op=mybir.AluOpType.is_ge, fill=fz,
                                base=0, pattern=[[0, 8], [-1, 128]],
                                channel_multiplier=1)
                        else:
                            nc.gpsimd.affine_select(
                                out=pv8, in_=pv8,
                                compare_op=mybir.AluOpType.is_ge, fill=fz,
                                base=-1, pattern=[[0, 8], [1, 128]],
                                channel_multiplier=-1)
                    else:
                        nc.vector.tensor_mul(p2, p2, mA if j == 0 else mB)
                return p2

            # ---- PV + normalize: flipped probs@[1|V] per half into one
            # output pair; one recip [64,1024]; four STTs scatter into attnT.
            attnT_v = attnT.rearrange("p (k t) -> p k t", k=8)

            def pv(mt, gg, p2s):
                qcol = mt * 128
                op = ps.tile([128, 1024], f32, tag="P", bufs=4, name="op")
                for h in range(2):
                    for j in range(3):
                        nc.tensor.matmul(
                            out=op[:, h * 512:(h + 1) * 512],
                            lhsT=v_view[:, mt + j, h, :],
                            rhs=p2s[j][:, h * 512:(h + 1) * 512],
                            start=(j == 0), stop=(j == 2))
                rc = attnp.tile([64, 1024], f32, tag="rc", bufs=2, name="rc")
                nc.vector.reciprocal_approx_fast(out=rc, in_=op[0:64, :])
                num = op[64:128, :].rearrange("p (h c e t) -> p h c e t",
                                              h=2, c=2, e=2)
                rcv = rc.rearrange("p (h c e t) -> p h c e t", h=2, c=2, e=2)
                for h in range(2):
                    k0 = 2 * gg + 4 * h
                    for e in range(2):
                        nc.vector.scalar_tensor_tensor(
                            out=attnT_v[64 * e:64 * e + 64, k0:k0 + 2,
                                        qcol:qcol + 128],
                            in0=num[:, h, :, e, :], scalar=1.0,
                            in1=rcv[:, h, :, e, :], op0=MUL, op1=MUL)

            # ---- out projection: accumulate both 512-col halves in one
            # PSUM pair; k-order follows PV-group completion order.
            def oproj(mt, split_tail=False):
                qcol = mt * 128
                o2 = ps.tile([128, 1024], f32, tag="P", bufs=4, name="o2")
                for k in (0, 1, 4, 5, 2, 3, 6, 7):
                    for n in range(2):
                        nc.tensor.matmul(
                            out=o2[:, n * 512:(n + 1) * 512],
                            lhsT=attnT[:, k * T + qcol:k * T + qcol + 128],
                            rhs=wo_sb[:, k * DIM + n * 512:
                                      k * DIM + (n + 1) * 512],
                            start=(k == 0), stop=(k == 7))
                out_t = attnp.tile([128, DIM], bf16, tag="outt", bufs=2,
                                   name="out_t")
                if split_tail:
                    nc.scalar.copy(out=out_t[:, 0:512], in_=o2[:, 0:512])
                    nc.sync.dma_start(out=out[qcol:qcol + 128, 0:512],
                                      in_=out_t[:, 0:512])
                    nc.vector.tensor_copy(out=out_t[:, 512:1024],
                                          in_=o2[:, 512:1024])
                    nc.sync.dma_start(out=out[qcol:qcol + 128, 512:1024],
                                      in_=out_t[:, 512:1024])
                else:
                    if mt % 2 == 0:
                        nc.scalar.copy(out=out_t, in_=o2)
                    else:
                        nc.vector.tensor_copy(out=out_t, in_=o2)
                    nc.sync.dma_start(out=out[qcol:qcol + 128, :], in_=out_t)

            # ---- head: Q m0-3 k-major (DMA-gated), then K, then V 0-2 ----
            q_sub((0, 1))
            q_sub((2, 3))
            k_proj()
            for ut in range(3):
                v_proj(ut)

            # ---- software-pipelined attention loop; Q m4-7 and V 3-9 are
            # PE filler in the first iterations (attention exps overlap the
            # remaining projection matmuls).
            fillers = {
                0: [lambda: q_sub((4, 5)), lambda: q_sub((6, 7)),
                    lambda: v_proj(3), lambda: v_proj(4)],
                1: [lambda: v_proj(5), lambda: v_proj(6)],
                2: [lambda: v_proj(7), lambda: v_proj(8)],
                3: [lambda: v_proj(9)],
            }
            prev_p2g1 = None
            prev = None
            for mt in range(8):
                last = (mt == 7)
                if last:
                    g1 = [scores_pair(mt, 1, j) for j in range(3)]
                g0 = [scores_pair(mt, 0, 0), scores_pair(mt, 0, 1)]
                if prev is not None:
                    pv(prev, 1, prev_p2g1)
                g0.append(scores_pair(mt, 0, 2))
                if prev is not None:
                    oproj(prev)
                for f in fillers.get(mt, ()):
                    f()
                if not last:
                    g1 = [scores_pair(mt, 1, j) for j in range(3)]
                pv(mt, 0, g0)
                prev_p2g1 = g1
                prev = mt
            pv(prev, 1, prev_p2g1)
            oproj(prev, split_tail=True)

    nc.compile()
    return nc


def _host_prep(x, Wq, bq, Wk, bk, Wv, bv, Wo, bo):
    import ml_dtypes
    bf16 = ml_dtypes.bfloat16

    def fold8(a, width):
        # [1024, width] -> [128, 8*width] with chunk k at cols k*width
        return np.ascontiguousarray(
            a.reshape(8, 128, width).transpose(1, 0, 2).reshape(128, 8 * width))

    # permute Wq columns so qT m-tile holds head m on partitions 0-63 and
    # head m+8 on partitions 64-127 (row-packed score matmuls)
    idx = np.empty(DIM, dtype=np.int64)
    for m in range(8):
        for j in range(128):
            h = m if j < 64 else m + 8
            idx[m * 128 + j] = h * D + (j % 64)
    wq_p = fold8(np.ascontiguousarray(Wq[:, idx]), DIM).astype(bf16)
    bq_p = bq[idx].astype(np.float32).reshape(8, 128).T.copy()  # (128, 8)

    # permute Wo rows to match the flipped-PV attnT layout:
    # attnT row r = k*128 + p with k = 2gg + 4half + c, e = p//64, d = p%64,
    # head h = 4gg + 8half + 2c + e, original row h*64 + d.
    oidx = np.empty(DIM, dtype=np.int64)
    for k in range(8):
        half, gg, c = k // 4, (k % 4) // 2, k % 2
        for p in range(128):
            e, d = p // 64, p % 64
            h = 4 * gg + 8 * half + 2 * c + e
            oidx[k * 128 + p] = h * D + d
    wo_p = fold8(np.ascontiguousarray(Wo[oidx, :]), DIM).astype(bf16)

    wk_p = fold8(np.ascontiguousarray(Wk), KVD).astype(bf16)
    wv_p = fold8(np.ascontiguousarray(Wv), KVD).astype(bf16)

    # band masks [128, (half,i=4, c=128)]: mA keeps key r >= query c (j=0),
    # mB keeps r < c (j=2); identical across the 8 head-blocks.
    r = np.arange(128)[:, None]
    c = np.arange(128)[None, :]
    mA_blk = (r >= c).astype(bf16)
    mB_blk = (r < c).astype(bf16)
    mA_p = np.tile(mA_blk, (1, 8)).astype(bf16)
    mB_p = np.tile(mB_blk, (1, 8)).astype(bf16)

    in_maps = []
    for core in range(NCORES):
        b, qt = core // QT, core % QT
        lo, hi = qt * T - HW, qt * T + T + HW
        xs = np.zeros((TH, DIM), dtype=np.float32)
        s0, s1 = max(lo, 0), min(hi, S)
        xs[s0 - lo:s1 - lo] = x[b, s0:s1]
        ind = np.zeros(TH, dtype=np.float32)
        ind[s0 - lo:s1 - lo] = 1.0
        # bkm[p, t] = bk[p] * ind[t]; bvm[p, (u, g, d)] = bv[g*64+d]*ind[u*128+p]
        bkm = (bk.astype(np.float32)[:, None] * ind[None, :]).astype(bf16)
        bvm = (ind.reshape(NU, 128).T[:, :, None, None] *
               bv.astype(np.float32).reshape(1, 1, 2, D)).reshape(
                   128, NU * 2 * D).astype(bf16)
        in_maps.append({
            "xTp": fold8(np.ascontiguousarray(xs.T).reshape(DIM, TH), TH).astype(bf16),
            "wqp": wq_p, "wkp": wk_p, "wvp": wv_p, "wop": wo_p,
            "bqc": bq_p, "bkmp": bkm, "bvmp": bvm,
            "mAp": mA_p, "mBp": mB_p,
        })
    return in_maps


def kernel(x, Wq, bq, Wk, bk, Wv, bv, Wo, bo):
    from concourse.bass_utils import run_bass_kernel_spmd

    x, Wq, bq, Wk, bk, Wv, bv, Wo, bo = (
        np.asarray(a, dtype=np.float32)
        for a in (x, Wq, bq, Wk, bk, Wv, bv, Wo, bo))
    nc = _build_nc()
    in_maps = _host_prep(x, Wq, bq, Wk, bk, Wv, bv, Wo, bo)
    res = run_bass_kernel_spmd(nc, in_maps, core_ids=list(range(NCORES)))
    out = np.empty((B, S, DIM), dtype=np.float32)
    for c in range(NCORES):
        b, qt = c // QT, c % QT
        out[b, qt * T:(qt + 1) * T] = res.results[c]["out"].astype(np.float32)
    out += bo  # output bias is purely additive after the last matmul
    return out


# revision 4
# speedup vs baseline: 1.0660x; 1.0660x over previous
"""Trainium2 Bass kernel for sliding-window GQA attention block (v4).

Reference computation (B=2, S=4096, DIM=1024, H=16 q-heads, KV=2 kv-heads,
D=64, W=256 window):
    q = x@Wq + bq ; k = x@Wk + bk ; v = x@Wv + bv        (GQA repeat kv x8)
    local attention: query t attends keys [t-128, t+128) (zero-padded edges,
    no 1/sqrt(d) scaling), softmax, out = probs@v
    y = out@Wo + bo
Sharding: 8 cores = batch(2) x seq-quarter(4); each core computes 1024 query
rows end-to-end from a 1280-row haloed x slice; bo added on host.

v4 structure (vs 146us v2 baseline):
  - epilogue constants are HOST inputs: bkm = outer(bk, ind), bvm =
    bv x ind, band masks mA/mB.  x halo columns are zero so projections
    are already zero there -> single tensor_add fuses bias + halo mask.
  - PSUM as [128,1024] bank-pairs; both kv-halves of a score j-chunk
    share a pair -> ONE exp ACTIVATE per chunk (the 352-cycle ACT fixed
    cost amortizes), one reciprocal per (mt,gg).
  - attnT k-tile order k = 4*gg + 2*half + c (host permutes Wo rows to
    match) makes the normalize a single 3D-AP STT per (mt,gg,e).
  - band masks on DVE (tensor_mul with host masks); gpsimd only issues
    DMA descriptors (its semaphore path is too slow for the p2 chain).
  - head: all projections (Q/K/V) run before the loop; Q m0-3 are
    emitted k-major so the PE streams behind the xT/wq DMAs, which are
    interleaved per-chunk across the three DMA queues.  The attention
    loop is then pure scores/PV/out-proj.
  - ACT exp table preloaded with a dummy activation at t=0.
"""

import functools
import numpy as np

B, S, DIM = 2, 4096, 1024
H, KV, D = 16, 2, 64
W, HW = 256, 128
NCORES = 8
QT = 4           # sequence quarters
T = S // QT      # 1024 query rows per core
TH = T + 2 * HW  # 1280 haloed rows
NU = TH // 128   # 10 key/value u-tiles
KVD = KV * D     # 128


@functools.lru_cache(maxsize=1)
def _build_nc():
    import concourse.bacc as bacc
    import concourse.tile as tile
    from concourse import mybir

    f32 = mybir.dt.float32
    bf16 = mybir.dt.bfloat16
    Exp = mybir.ActivationFunctionType.Exp
    Identity = mybir.ActivationFunctionType.Identity
    MUL = mybir.AluOpType.mult

    nc = bacc.Bacc("TRN2", target_bir_lowering=False, debug=False)

    xTp = nc.dram_tensor("xTp", [128, 8 * TH], bf16, kind="ExternalInput")
    wqp = nc.dram_tensor("wqp", [128, 8 * DIM], bf16, kind="ExternalInput")
    wkp = nc.dram_tensor("wkp", [128, 8 * KVD], bf16, kind="ExternalInput")
    wvp = nc.dram_tensor("wvp", [128, 8 * KVD], bf16, kind="ExternalInput")
    wop = nc.dram_tensor("wop", [128, 8 * DIM], bf16, kind="ExternalInput")
    bqc = nc.dram_tensor("bqc", [128, 8], f32, kind="ExternalInput")
    bkmp = nc.dram_tensor("bkmp", [128, TH], bf16, kind="ExternalInput")
    bvmp = nc.dram_tensor("bvmp", [128, TH], bf16, kind="ExternalInput")
    mAp = nc.dram_tensor("mAp", [128, 1024], bf16, kind="ExternalInput")
    mBp = nc.dram_tensor("mBp", [128, 1024], bf16, kind="ExternalInput")
    out = nc.dram_tensor("out", [T, DIM], bf16, kind="ExternalOutput")

    with tile.TileContext(nc) as tc:
        with tc.tile_pool(name="const", bufs=1) as const, \
             tc.tile_pool(name="w", bufs=1) as wpool, \
             tc.tile_pool(name="act", bufs=1) as actp, \
             tc.tile_pool(name="attn", bufs=2) as attnp, \
             tc.tile_pool(name="ps", bufs=2, space="PSUM") as ps:

            # ---- ACT exp-table preload at t=0 (overlaps the DMA phase)
            dum = const.tile([1, 2], f32, tag="dum")
            dumo = const.tile([1, 2], f32, tag="dumo")
            nc.vector.memset(dum, 0.0)
            nc.scalar.activation(out=dumo, in_=dum, func=Exp)

            # ---- SBUF tiles -----------------------------------------------
            xT_sb = wpool.tile([128, 8 * TH], bf16, tag="xT")
            wq_sb = wpool.tile([128, 8 * DIM], bf16, tag="wq")
            wk_sb = wpool.tile([128, 8 * KVD], bf16, tag="wk")
            wv_sb = wpool.tile([128, 8 * KVD], bf16, tag="wv")
            wo_sb = wpool.tile([128, 8 * DIM], bf16, tag="wo")
            bq_sb = const.tile([128, 8], f32, tag="bq")
            bkm_sb = const.tile([128, TH], bf16, tag="bkm")
            bvm_sb = const.tile([128, TH], bf16, tag="bvm")
            mA = const.tile([128, 1024], bf16, tag="mA")
            mB = const.tile([128, 1024], bf16, tag="mB")

            # ---- DMA schedule: wk/wv first (tiny, K/V never wait), then
            # xT/wq chunk-interleaved across the three DMA queues in the
            # order the k-major Q matmuls consume them; consts next; wo
            # (first needed ~45us in) last, split across queues.
            sq, sc, gq = nc.sync, nc.scalar, nc.gpsimd
            qs = [sq, sc, gq]
            sc.dma_start(out=wk_sb, in_=wkp[:, :])
            gq.dma_start(out=wv_sb, in_=wvp[:, :])
            for k in range(8):
                qs[(2 * k) % 3].dma_start(
                    out=xT_sb[:, k * TH:(k + 1) * TH],
                    in_=xTp[:, k * TH:(k + 1) * TH])
                qs[(2 * k + 1) % 3].dma_start(
                    out=wq_sb[:, k * DIM:(k + 1) * DIM],
                    in_=wqp[:, k * DIM:(k + 1) * DIM])
            sc.dma_start(out=bq_sb, in_=bqc[:, :])
            sq.dma_start(out=bkm_sb, in_=bkmp[:, :])
            gq.dma_start(out=bvm_sb, in_=bvmp[:, :])
            sq.dma_start(out=mA, in_=mAp[:, :])
            gq.dma_start(out=mB, in_=mBp[:, :])
            for k in range(8):
                qs[k % 3].dma_start(out=wo_sb[:, k * DIM:(k + 1) * DIM],
                                    in_=wop[:, k * DIM:(k + 1) * DIM])

            # ---- activations / attention SBUF -----------------------------
            qT_sb = [actp.tile([128, 4 * T], bf16, tag=f"qT{g}", name=f"qT{g}")
                     for g in range(2)]
            kT_sb = actp.tile([128, TH], bf16, tag="kT")
            v_sb = actp.tile([128, NU * 256], bf16, tag="V")
            v_view = v_sb.rearrange("p (u g c) -> p u g c", u=NU, g=2)
            nc.vector.memset(v_view[:, :, :, 0:64], 1.0)
            bvm_v = bvm_sb.rearrange("p (u g d) -> p u g d", u=NU, g=2)
            attnT = actp.tile([128, 8 * T], bf16, tag="attnT")
            attnT_v = attnT.rearrange("p (k t) -> p k t", k=8)
            qvs = [qT_sb[g].rearrange("p (i t) -> p i t", i=4) for g in range(2)]

            # ---- Q projection: two m-tiles k-inner per call; the first two
            # calls' k-loops stream directly behind the xT/wq DMAs.  Bias
            # copy into qT: even m on ACT, odd m on DVE.
            def q_sub(ms, interleaved=False):
                prs = {m: ps.tile([128, 1024], f32, tag="SP", bufs=2,
                                  name=f"qp{m}") for m in ms}
                for k in range(8):
                    for m in ms:
                        for n in range(2):
                            nc.tensor.matmul(
                                out=prs[m][:, n * 512:(n + 1) * 512],
                                lhsT=wq_sb[:, k * DIM + m * 128:
                                           k * DIM + (m + 1) * 128],
                                rhs=xT_sb[:, k * TH + HW + n * 512:
                                          k * TH + HW + (n + 1) * 512],
                                start=(k == 0), stop=(k == 7))
                for m in ms:
                    dst = qT_sb[m // 4][:, (m % 4) * T:(m % 4) * T + 1024]
                    if m % 2 == 0:
                        nc.scalar.activation(out=dst, in_=prs[m],
                                             func=Identity,
                                             bias=bq_sb[:, m:m + 1], scale=1.0)
                    else:
                        nc.vector.tensor_scalar_add(out=dst, in0=prs[m],
                                                    scalar1=bq_sb[:, m:m + 1])

            # sg0+sg1 interleaved k-major: 4 PSUM pairs live, PE consumes
            # each (xT[k], wq[k]) chunk-pair with 16 matmuls as it lands.
            def q_head():
                prs = {m: ps.tile([128, 1024], f32,
                                  tag=("SP" if m < 2 else "OP"), bufs=2,
                                  name=f"qp{m}") for m in range(4)}
                for k in range(8):
                    for m in range(4):
                        for n in range(2):
                            nc.tensor.matmul(
                                out=prs[m][:, n * 512:(n + 1) * 512],
                                lhsT=wq_sb[:, k * DIM + m * 128:
                                           k * DIM + (m + 1) * 128],
                                rhs=xT_sb[:, k * TH + HW + n * 512:
                                          k * TH + HW + (n + 1) * 512],
                                start=(k == 0), stop=(k == 7))
                for m in range(4):
                    dst = qT_sb[0][:, m * T:m * T + 1024]
                    if m % 2 == 0:
                        nc.scalar.activation(out=dst, in_=prs[m],
                                             func=Identity,
                                             bias=bq_sb[:, m:m + 1], scale=1.0)
                    else:
                        nc.vector.tensor_scalar_add(out=dst, in0=prs[m],
                                                    scalar1=bq_sb[:, m:m + 1])

            def k_proj():
                kp01 = ps.tile([128, 1024], f32, tag="SP", bufs=2, name="kp01")
                kp2 = ps.tile([128, 1024], f32, tag="OP", bufs=2, name="kp2")
                for k in range(8):
                    for c in range(2):
                        nc.tensor.matmul(
                            out=kp01[:, c * 512:(c + 1) * 512],
                            lhsT=wk_sb[:, k * KVD:(k + 1) * KVD],
                            rhs=xT_sb[:, k * TH + c * 512:k * TH + (c + 1) * 512],
                            start=(k == 0), stop=(k == 7))
                    nc.tensor.matmul(
                        out=kp2[:, 0:256],
                        lhsT=wk_sb[:, k * KVD:(k + 1) * KVD],
                        rhs=xT_sb[:, k * TH + 1024:k * TH + 1280],
                        start=(k == 0), stop=(k == 7))
                nc.vector.tensor_add(out=kT_sb[:, 0:1024], in0=kp01,
                                     in1=bkm_sb[:, 0:1024])
                nc.vector.tensor_add(out=kT_sb[:, 1024:1280], in0=kp2[:, 0:256],
                                     in1=bkm_sb[:, 1024:1280])

            def v_proj(ut):
                vp = ps.tile([128, 1024], f32, tag="OP", bufs=2, name="vp")
                for k in range(8):
                    nc.tensor.matmul(
                        out=vp[:, 0:128],
                        lhsT=xT_sb[:, k * TH + ut * 128:k * TH + (ut + 1) * 128],
                        rhs=wv_sb[:, k * KVD:(k + 1) * KVD],
                        start=(k == 0), stop=(k == 7))
                nc.vector.tensor_add(
                    out=v_view[:, ut, :, 64:128],
                    in0=vp[:, 0:128].rearrange("p (g c) -> p g c", g=2),
                    in1=bvm_v[:, ut])

            # ---- scores j-chunk: both kv-halves into one PSUM pair, one
            # exp over [128,1024]; band mask (j=0/2) via DVE tensor_mul.
            def scores_pair(mt, gg, j):
                qcol = mt * 128
                sp = ps.tile([128, 1024], f32, tag="SP", bufs=2, name="sp")
                for h in range(2):
                    nc.tensor.matmul(
                        out=sp[:, h * 512:(h + 1) * 512],
                        lhsT=kT_sb[h * 64:(h + 1) * 64,
                                   qcol + j * 128:qcol + (j + 1) * 128],
                        rhs=qvs[gg][h * 64:(h + 1) * 64, :, qcol:qcol + 128],
                        start=True, stop=True,
                        tile_position=(64 * h, 0))
                p2 = attnp.tile([128, 1024], bf16, tag="p2", bufs=10, name="p2")
                nc.scalar.activation(out=p2, in_=sp, func=Exp)
                if j == 0:
                    nc.vector.tensor_mul(p2, p2, mA)
                elif j == 2:
                    nc.vector.tensor_mul(p2, p2, mB)
                return p2

            # ---- PV + normalize: probs@[1|V] per half into one output
            # pair; one recip [64,1024]; one STT per e writes the four
            # k'-tiles (k' = 4gg + 2h + c, contiguous) of attnT.
            def pv(mt, gg, p2s):
                qcol = mt * 128
                op = ps.tile([128, 1024], f32, tag="OP", bufs=2, name="op")
                for h in range(2):
                    for j in range(3):
                        nc.tensor.matmul(
                            out=op[:, h * 512:(h + 1) * 512],
                            lhsT=v_view[:, mt + j, h, :],
                            rhs=p2s[j][:, h * 512:(h + 1) * 512],
                            start=(j == 0), stop=(j == 2))
                rc = attnp.tile([64, 1024], f32, tag="rc", bufs=2, name="rc")
                nc.vector.reciprocal_approx_fast(out=rc, in_=op[0:64, :])
                # free dim of op[64:128] is (h, c, e, t); for fixed e the
                # (h, c) dims are stride 512/256 -> merge into one 4-wide
                # dim matching attnT k'-tiles 4gg..4gg+3 (stride T).
                num = op[64:128, :].rearrange("p (hc e t) -> p hc e t",
                                              hc=4, e=2)
                rcv = rc.rearrange("p (hc e t) -> p hc e t", hc=4, e=2)
                for e in range(2):
                    nc.vector.scalar_tensor_tensor(
                        out=attnT_v[64 * e:64 * e + 64, 4 * gg:4 * gg + 4,
                                    qcol:qcol + 128],
                        in0=num[:, :, e, :], scalar=1.0,
                        in1=rcv[:, :, e, :], op0=MUL, op1=MUL)

            # ---- out projection: one PSUM pair; gg0 k'-tiles (0-3) first
            # so the accumulation overlaps the second PV group's normalize.
            def oproj(mt):
                qcol = mt * 128
                o2 = ps.tile([128, 1024], f32, tag="OP", bufs=2, name="o2")
                for k in range(8):
                    for n in range(2):
                        nc.tensor.matmul(
                            out=o2[:, n * 512:(n + 1) * 512],
                            lhsT=attnT[:, k * T + qcol:k * T + qcol + 128],
                            rhs=wo_sb[:, k * DIM + n * 512:
                                      k * DIM + (n + 1) * 512],
                            start=(k == 0), stop=(k == 7))
                out_t = attnp.tile([128, DIM], bf16, tag="outt", bufs=2,
                                   name="out_t")
                nc.scalar.copy(out=out_t[:, 0:512], in_=o2[:, 0:512])
                nc.sync.dma_start(out=out[qcol:qcol + 128, 0:512],
                                  in_=out_t[:, 0:512])
                nc.vector.tensor_copy(out=out_t[:, 512:1024],
                                      in_=o2[:, 512:1024])
                nc.sync.dma_start(out=out[qcol:qcol + 128, 512:1024],
                                  in_=out_t[:, 512:1024])

            # ---- head: all projections before the loop ---------------------
            q_head()          # m0-3 k-major behind the DMA stream
            k_proj()
            q_sub((4, 5))
            q_sub((6, 7))
            for ut in range(NU):
                v_proj(ut)

            # ---- attention loop (pure): software-pipelined per qtile ------
            prev_p2g1 = None
            prev = None
            for mt in range(8):
                last = (mt == 7)
                if last:
                    g1 = [scores_pair(mt, 1, j) for j in range(3)]
                g0 = [scores_pair(mt, 0, 0), scores_pair(mt, 0, 1)]
                if prev is not None:
                    pv(prev, 1, prev_p2g1)
                g0.append(scores_pair(mt, 0, 2))
                if prev is not None:
                    oproj(prev)
                if not last:
                    g1 = [scores_pair(mt, 1, j) for j in range(3)]
                pv(mt, 0, g0)
                prev_p2g1 = g1
                prev = mt
            pv(prev, 1, prev_p2g1)
            oproj(prev)

    nc.compile()
    return nc


def _host_prep(x, Wq, bq, Wk, bk, Wv, bv, Wo, bo):
    import ml_dtypes
    bf16 = ml_dtypes.bfloat16

    def fold8(a, width):
        # [1024, width] -> [128, 8*width] with chunk k at cols k*width
        return np.ascontiguousarray(
            a.reshape(8, 128, width).transpose(1, 0, 2).reshape(128, 8 * width))

    # permute Wq columns so qT m-tile holds head m on partitions 0-63 and
    # head m+8 on partitions 64-127 (row-packed score matmuls)
    idx = np.empty(DIM, dtype=np.int64)
    for m in range(8):
        for j in range(128):
            h = m if j < 64 else m + 8
            idx[m * 128 + j] = h * D + (j % 64)
    wq_p = fold8(np.ascontiguousarray(Wq[:, idx]), DIM).astype(bf16)
    bq_p = bq[idx].astype(np.float32).reshape(8, 128).T.copy()  # (128, 8)

    # permute Wo rows to match the flipped-PV attnT layout:
    # attnT row r = k*128 + p with k = 4gg + 2half + c, e = p//64, d = p%64,
    # head h = 4gg + 8half + 2c + e, original row h*64 + d.
    oidx = np.empty(DIM, dtype=np.int64)
    for k in range(8):
        gg, half, c = k // 4, (k % 4) // 2, k % 2
        for p in range(128):
            e, d = p // 64, p % 64
            h = 4 * gg + 8 * half + 2 * c + e
            oidx[k * 128 + p] = h * D + d
    wo_p = fold8(np.ascontiguousarray(Wo[oidx, :]), DIM).astype(bf16)

    wk_p = fold8(np.ascontiguousarray(Wk), KVD).astype(bf16)
    wv_p = fold8(np.ascontiguousarray(Wv), KVD).astype(bf16)

    # band masks [128 keys, (half, i=4, c=128)]: mA keeps key r >= query c
    # (j=0), mB keeps r < c (j=2); identical across the 8 head-blocks.
    r = np.arange(128)[:, None]
    c = np.arange(128)[None, :]
    mA_p = np.tile((r >= c), (1, 8)).astype(bf16)
    mB_p = np.tile((r < c), (1, 8)).astype(bf16)

    in_maps = []
    for core in range(NCORES):
        b, qt = core // QT, core % QT
        lo, hi = qt * T - HW, qt * T + T + HW
        xs = np.zeros((TH, DIM), dtype=np.float32)
        s0, s1 = max(lo, 0), min(hi, S)
        xs[s0 - lo:s1 - lo] = x[b, s0:s1]
        ind = np.zeros(TH, dtype=np.float32)
        ind[s0 - lo:s1 - lo] = 1.0
        # bkm[p, t] = bk[p]*ind[t]; bvm[p, (u,g,d)] = bv[g*64+d]*ind[u*128+p]
        bkm = (bk.astype(np.float32)[:, None] * ind[None, :]).astype(bf16)
        bvm = (ind.reshape(NU, 128).T[:, :, None, None] *
               bv.astype(np.float32).reshape(1, 1, 2, D)).reshape(
                   128, NU * 2 * D).astype(bf16)
        in_maps.append({
            "xTp": fold8(np.ascontiguousarray(xs.T), TH).astype(bf16),
            "wqp": wq_p, "wkp": wk_p, "wvp": wv_p, "wop": wo_p,
            "bqc": bq_p, "bkmp": bkm, "bvmp": bvm,
            "mAp": mA_p, "mBp": mB_p,
        })
    return in_maps


def kernel(x, Wq, bq, Wk, bk, Wv, bv, Wo, bo):
    from concourse.bass_utils import run_bass_kernel_spmd

    x, Wq, bq, Wk, bk, Wv, bv, Wo, bo = (
        np.asarray(a, dtype=np.float32)
        for a in (x, Wq, bq, Wk, bk, Wv, bv, Wo, bo))
    nc = _build_nc()
    in_maps = _host_prep(x, Wq, bq, Wk, bk, Wv, bv, Wo, bo)
    res = run_bass_kernel_spmd(nc, in_maps, core_ids=list(range(NCORES)))
    out = np.empty((B, S, DIM), dtype=np.float32)
    for c in range(NCORES):
        b, qt = c // QT, c % QT
        out[b, qt * T:(qt + 1) * T] = res.results[c]["out"].astype(np.float32)
    out += bo  # output bias is purely additive after the last matmul
    return out
